# revision 20
# baseline (speedup 1.0000x reference)
"""DLRM-style embedding lookup kernel for 8 TRN2 NeuronCores.

Strategy: 4 table-shards x 2 row-shards. Core c = (varlen table k=c//2,
row half h=c%2) performs the 50-id varlen lookup+pool for its table over
half the batch on device; the dim-1 sparse-table lookups, the 13-dim
dense matvec, and ~0.02% placement-overflow fixups are exact host-side
terms added at assembly (the SWDGE dma_gather path is not usable under
this runtime, so those 16 lookups/row stay on host as in the original).

Device pipeline per core: bf16-pair varlen tables (map0 ‖ map1
encodings, 8192 u32/partition in SBUF). The host places each id into a
parity-free bin grid (8 groups x 7 slots, 2-choice + cuckoo eviction;
group = top-3 or low-3 id bits selects the map). Per 1024-row block: one
k-major ap_gather (7168 idxs — the gpsimd gather is the critical path at
~27ns/idx, so fewer fetched u32/row than the older 16-bin parity grid is
a direct win), PE matmul expands host-sent [1, c, c^2] code rows into
(code - partition)^2 per bf16 HALF in PSUM (codes are per-half: the
half not holding the slot-id carries the pad code), DVE builds the
(z == 0) keep-mask and multiplies it in place, and accumulating
ones-matmuls pool both halves of each slot into PSUM (region-major so
each PSUM bank has one open accumulation group at a time).
"""
import sys
sys.path.insert(0, "/opt/trn_rl_repo")

import numpy as np
import ml_dtypes
from contextlib import ExitStack

B = 65536
N_SPARSE = 16
N_VARLEN = 4
MAXLEN = 50
N_DENSE = 13
VOCAB = 1_000_000

RC = B // 2            # rows per core (2 row-shards)
NG = 8                 # partition groups == bins per row
KPB = 7                # u32 slots per (row, group) bin
RB = 1024              # rows per device block
NB = RC // RB          # 32 blocks
BLK = KPB * RB         # gather idxs per group per block = 7168
STREAM = KPB * RC      # 229376 per group
NE_TBL = 8192          # u32 per partition (map0 + map1)
PADC = 16              # pad code (never matches partition 0..15)

_compiled = {}
_last_res = None


# ---------------------------------------------------------------- device ---
def _build_nc(repeat=1):
    """repeat > 1 re-runs the whole per-block pipeline (used only by the
    timing harness to measure per-execution time as a slope in reps,
    cancelling the large axon dispatch overhead)."""
    import concourse.bass as bass
    import concourse.tile as tile
    from concourse import bacc, mybir

    nc = bacc.Bacc("TRN2", target_bir_lowering=False, debug=False)
    dt = mybir.dt
    tbl = nc.dram_tensor("tbl", [128, NE_TBL], dt.uint32, kind="ExternalInput").ap()
    didx = nc.dram_tensor("didx", [128, STREAM // 16], dt.int16, kind="ExternalInput").ap()
    cmat = nc.dram_tensor("cmat", [17, 2 * STREAM], dt.bfloat16, kind="ExternalInput").ap()
    w17 = nc.dram_tensor("w17", [17, 128], dt.bfloat16, kind="ExternalInput").ap()
    outv = nc.dram_tensor("outv", [NB, RB], dt.float32, kind="ExternalOutput").ap()

    CH = 512                        # bf16 halves per z / copy_pred chunk
    NCH = 2 * BLK // CH             # 28 chunks per block
    with tile.TileContext(nc) as tc:
        with ExitStack() as ctx:
            pconst = ctx.enter_context(tc.tile_pool(name="const", bufs=1))
            pidx = ctx.enter_context(tc.tile_pool(name="idx", bufs=2))
            pcm = ctx.enter_context(tc.tile_pool(name="cm", bufs=2))
            pgat = ctx.enter_context(tc.tile_pool(name="gat", bufs=2))
            pzbf = ctx.enter_context(tc.tile_pool(name="zbf", bufs=3))
            pstage = ctx.enter_context(tc.tile_pool(name="stage", bufs=2))
            pz = ctx.enter_context(tc.tile_pool(name="pz", bufs=3, space="PSUM"))
            po = ctx.enter_context(tc.tile_pool(name="po", bufs=2, space="PSUM"))

            t_tbl = pconst.tile([128, NE_TBL], dt.uint32)
            nc.sync.dma_start(t_tbl[:], tbl[:])
            t_w17 = pconst.tile([17, 128], dt.bfloat16)
            nc.sync.dma_start(t_w17[:], w17[:])
            t_ones = pconst.tile([128, 1], dt.bfloat16)
            nc.vector.memset(t_ones[:], 1.0)
            t_zero = pconst.tile([128, CH], dt.bfloat16)
            nc.vector.memset(t_zero[:], 0)

            def input_dma(b):
                t_di = pidx.tile([128, BLK // 16], dt.int16, tag="di")
                nc.sync.dma_start(t_di[:], didx[:, b * (BLK // 16):(b + 1) * (BLK // 16)])
                t_cm = pcm.tile([17, 2 * BLK], dt.bfloat16, tag="cm")
                nc.sync.dma_start(t_cm[:], cmat[:, b * 2 * BLK:(b + 1) * 2 * BLK])
                return t_di, t_cm

            for b in [b for _ in range(repeat) for b in range(NB)]:
                t_di, t_cm = input_dma(b)
                t_gat = pgat.tile([128, BLK, 1], dt.uint32, tag="g")
                nc.gpsimd.ap_gather(t_gat[:], t_tbl[:].unsqueeze(-1), t_di[:],
                                    channels=128, num_elems=NE_TBL, d=1, num_idxs=BLK)

                gat_bf = t_gat[:].squeeze(-1).bitcast(dt.bfloat16)  # [128, 2*BLK]
                # half-stream position f = s*2048 + rl*2 + h; chunk c of 512
                # covers (s = c//4, rl in [256*(c%4), 256*(c%4)+256), h in 0..1)
                view = gat_bf.rearrange("p (c x two) -> p c two x", c=NCH, two=2)

                t_po = po.tile([1, RB], dt.float32, tag="po")

                def emit_z(c):
                    t_pz = pz.tile([128, CH], dt.float32, tag="pz")
                    nc.tensor.matmul(t_pz[:], t_w17[:], t_cm[:, c * CH:(c + 1) * CH],
                                     start=True, stop=True)
                    # m = (z == 0) as bf16 1.0/0.0 — z is an exact integer
                    t_zb = pzbf.tile([128, CH], dt.bfloat16, tag="zb")
                    nc.vector.tensor_scalar(t_zb[:], t_pz[:], 0.0, None,
                                            mybir.AluOpType.is_equal)
                    return t_zb

                # process chunks region-major (all 7 slots of one po
                # 256-col region before the next) so each PSUM bank has at
                # most one open accumulation group at a time
                order = [s * 4 + rlb for rlb in range(4) for s in range(KPB)]
                zbuf = {c: emit_z(c) for c in order[:2]}
                for i, c in enumerate(order):
                    if i + 2 < NCH:
                        zbuf[order[i + 2]] = emit_z(order[i + 2])
                    t_zb = zbuf.pop(c)
                    nc.vector.tensor_tensor(
                        out=gat_bf[:, c * CH:(c + 1) * CH],
                        in0=gat_bf[:, c * CH:(c + 1) * CH],
                        in1=t_zb[:], op=mybir.AluOpType.mult)
                    s, rlb = c // 4, c % 4
                    first = (s == 0)
                    last = (s == KPB - 1)
                    for h in range(2):
                        nc.tensor.matmul(t_po[:, rlb * 256:(rlb + 1) * 256],
                                         t_ones[:], view[:, c, h, :],
                                         start=(first and h == 0),
                                         stop=(last and h == 1))
                t_st = pstage.tile([1, RB], dt.float32, tag="vst")
                nc.scalar.copy(t_st[:], t_po[:])
                nc.sync.dma_start(outv[b:b + 1, :], t_st[:])
    nc.compile()
    return nc


# ------------------------------------------------------------------ host ---
def _pack_u32(bf):  # bf16 [..., 2n] -> u32 [..., n]
    u = np.ascontiguousarray(bf).view(np.uint16).astype(np.uint32)
    return (u[..., 0::2] | (u[..., 1::2] << 16)).astype(np.uint32)


def _table_image(emb_varlen_k):
    """[128, 8192] u32: map0 ‖ map1 pair-packed bf16."""
    bf = ml_dtypes.bfloat16
    v = np.concatenate([emb_varlen_k, np.zeros(2**20 - VOCAB, emb_varlen_k.dtype)])
    v = v.astype(bf)
    m0 = v.reshape(8, 16, 8192).reshape(128, 8192)
    m1 = v.reshape(8192, 16, 8).transpose(2, 1, 0).reshape(128, 8192)
    return _pack_u32(np.concatenate([m0, m1], axis=1))


def _enc(ids, use_map1):
    """per-id (epair, code q, live half) for the chosen map. ids >= 0.
    map0: entry = (id>>13 partition, pair (id>>1)&4095, half id&1)
    map1: entry = (partition (id&7)*16 + (id>>3)&15, pair id>>8, half (id>>7)&1)
    """
    e0 = ((ids >> 1) & 4095).astype(np.int16)
    c0 = ((ids >> 13) & 15).astype(np.int16)
    h0 = (ids & 1).astype(np.int16)
    e1 = (4096 + (ids >> 8)).astype(np.int16)
    c1 = ((ids >> 3) & 15).astype(np.int16)
    h1 = ((ids >> 7) & 1).astype(np.int16)
    return (np.where(use_map1, e1, e0), np.where(use_map1, c1, c0),
            np.where(use_map1, h1, h0))


def _place_core(vl):
    """vl [R, 50] varlen ids. Parity-free 2-choice + cuckoo into a
    [R, 8 bins, 7 slots] grid. Bin g==(id>>17)&7 -> map0 placement;
    bin g==id&7 -> map1 (if both, map0). Returns grid_id [R,8,7] int64
    (-1 empty), grid_m1 [R,8,7] bool, overflow (rows, ids)."""
    R = vl.shape[0]
    rows = np.arange(R)
    cnt = np.zeros((R, NG), np.int32)
    grid_id = np.full((R, NG, KPB), -1, np.int64)
    placed = np.zeros((R, 50), bool)
    valid_all = np.zeros((R, 50), bool)
    ids_all = vl.astype(np.int64)
    b0s = ((ids_all >> 17) & 7).astype(np.int32)
    b1s = (ids_all & 7).astype(np.int32)

    for j in range(50):
        ids = ids_all[:, j]
        valid = ids > 0
        b0 = b0s[:, j]; b1 = b1s[:, j]
        valid_all[:, j] = valid
        c0 = cnt[rows, b0]; c1 = cnt[rows, b1]
        use0 = c0 <= c1
        ba = np.where(use0, b0, b1); bb = np.where(use0, b1, b0)
        sa = cnt[rows, ba]
        ok_a = valid & (sa < KPB)
        ra = rows[ok_a]
        grid_id[ra, ba[ok_a], sa[ok_a]] = ids[ok_a]
        cnt[ra, ba[ok_a]] += 1
        placed[ok_a, j] = True
        rem = valid & ~ok_a
        sb = cnt[rows, bb]
        ok_b = rem & (sb < KPB)
        rb = rows[ok_b]
        grid_id[rb, bb[ok_b], sb[ok_b]] = ids[ok_b]
        cnt[rb, bb[ok_b]] += 1
        placed[ok_b, j] = True

    # vectorized cuckoo eviction: rows independent; resolve one overflow
    # item per row per round.
    ovf = valid_all & ~placed
    for _ in range(8):
        has = ovf.any(axis=1)
        if not has.any():
            break
        r_sel = np.nonzero(has)[0]
        j_sel = ovf[r_sel].argmax(axis=1)
        ids = ids_all[r_sel, j_sel]
        bopt = np.stack([b0s[r_sel, j_sel], b1s[r_sel, j_sel]], 1)
        done = np.zeros(len(r_sel), bool)
        for bi in range(2):
            b = bopt[:, bi]
            for s in range(KPB):
                occ = grid_id[r_sel, b, s]
                o_b0 = ((occ >> 17) & 7).astype(np.int32)
                o_b1 = (occ & 7).astype(np.int32)
                alt = np.where(o_b0 == b, o_b1, o_b0)
                can = (~done) & (occ >= 0) & (cnt[r_sel, alt] < KPB)
                rr = r_sel[can]
                t = cnt[rr, alt[can]]
                grid_id[rr, alt[can], t] = occ[can]
                cnt[rr, alt[can]] += 1
                grid_id[rr, b[can], s] = ids[can]
                ovf[rr, j_sel[can]] = False
                done |= can
            if done.all():
                break

    orows, ojs = np.nonzero(ovf)
    g = np.arange(NG)[None, :, None]
    gid = grid_id
    is_b0 = ((gid >> 17) & 7) == g
    grid_m1 = (gid >= 0) & ~is_b0
    return grid_id, grid_m1, (orows, ids_all[orows, ojs])


def prepare_in_maps(sparse_ids, varlen_ids, dense_vals, emb_sparse, emb_varlen,
                    dense_weight):
    sparse_ids = np.asarray(sparse_ids)
    varlen_ids = np.asarray(varlen_ids)
    dense_vals = np.asarray(dense_vals, np.float32)
    emb_sparse = np.asarray(emb_sparse, np.float32)
    emb_varlen = np.asarray(emb_varlen, np.float32)
    dense_weight = np.asarray(dense_weight, np.float32)
    bf = ml_dtypes.bfloat16

    w17 = np.zeros((17, 128), np.float32)
    p16 = np.arange(128) % 16
    pg = np.arange(128) // 16
    w17[0] = (p16 ** 2)
    for g in range(NG):
        w17[1 + 2 * g] = np.where(pg == g, -2.0 * p16, 0.0)
        w17[2 + 2 * g] = np.where(pg == g, 1.0, 0.0)

    in_maps = []
    fixups = np.zeros(B, np.float64)
    for c in range(8):
        k, h = c // 2, c % 2
        r0 = h * RC
        vl = varlen_ids[r0:r0 + RC, k, :]
        gid, gm1, (fr, fi) = _place_core(vl)
        if len(fr):
            np.add.at(fixups, r0 + fr,
                      emb_varlen[k][fi].astype(bf).astype(np.float64))

        ge, gq, gh = _enc(np.maximum(gid, 0), gm1)
        empty = gid < 0
        # empty slots gather a masked don't-care value, so place them at
        # fixed quantile anchors instead of 0: after the sort below, every
        # row's slot-s offset then tracks the s-th quantile of [0, 8192)
        # regardless of bin occupancy, keeping the paired-read address
        # deltas across consecutive rows small (e=0 clumps are no better
        # than random — see the const-idx probe)
        anchors = ((2 * np.arange(KPB) + 1) * (NE_TBL // (2 * KPB))).astype(np.int16)
        ge = np.where(empty, anchors[None, None, :], ge).astype(np.int16)
        # sort each bin's slots by table offset: slot s then holds the s-th
        # order statistic, so consecutive rows' same-slot offsets are
        # quantile-aligned — the gather ucode pairs reads two idxs apart in
        # the row-minor stream, and nearby addresses measure ~20% faster
        # than random on HW (any slot permutation is valid)
        srt = np.argsort(ge, axis=2, kind="stable")
        ge = np.take_along_axis(ge, srt, axis=2)
        gq = np.take_along_axis(gq, srt, axis=2)
        gh = np.take_along_axis(gh, srt, axis=2)
        empty = np.take_along_axis(empty, srt, axis=2)
        # per-half codes [R, 8, 7, 2]: live half gets q, other half PADC
        codes = np.full(gid.shape + (2,), PADC, np.int16)
        live = np.where(empty, PADC, gq).astype(np.int16)
        hsel = np.clip(gh, 0, 1).astype(np.int64)
        ridx, gidx, sidx = np.indices(gid.shape)
        codes[ridx, gidx, sidx, hsel] = live
        codes[empty] = PADC

        # [R,8,7] -> stream [g, pos]: pos = blk*7168 + s*1024 + rl
        def to_stream(a):
            x = a.reshape(NB, RB, NG, KPB)          # blk, rl, g, s
            x = x.transpose(2, 0, 3, 1)             # g, blk, s, rl
            return np.ascontiguousarray(x).reshape(NG, STREAM)

        estream = to_stream(ge)

        # codes [R,8,7,2] -> [g, blk, s, rl, h] -> [g, 2*STREAM]
        xc = codes.reshape(NB, RB, NG, KPB, 2).transpose(2, 0, 3, 1, 4)
        cstream = np.ascontiguousarray(xc).reshape(NG, 2 * STREAM).astype(np.float32)

        didx = np.empty((128, STREAM // 16), np.int16)
        for g in range(NG):
            t = estream[g].reshape(NB, BLK // 16, 16).transpose(0, 2, 1)
            didx[16 * g:16 * g + 16] = t.transpose(1, 0, 2).reshape(16, STREAM // 16)
        cmat = np.empty((17, 2 * STREAM), np.float32)
        cmat[0] = 1.0
        for g in range(NG):
            cmat[1 + 2 * g] = cstream[g]
            cmat[2 + 2 * g] = cstream[g] ** 2

        in_maps.append(dict(
            tbl=_table_image(emb_varlen[k]),
            didx=didx,
            cmat=cmat.astype(bf),
            w17=w17.astype(bf)))

    # host-side terms: sparse lookups + dense matvec (exact f64)
    tbl_s = np.arange(N_SPARSE)[None, :]
    sparse = emb_sparse.astype(np.float64)[tbl_s, sparse_ids].sum(axis=1)
    dense = (dense_vals.astype(np.float64) @ dense_weight.astype(np.float64)).reshape(B)
    return in_maps, fixups + sparse + dense


def assemble_output(results, extra):
    out = np.zeros(B, np.float64)
    for c in range(8):
        k, h = c // 2, c % 2
        r0 = h * RC
        out[r0:r0 + RC] += results[c]["outv"].reshape(RC).astype(np.float64)
    out += extra
    return out.astype(np.float32).reshape(B, 1)


def kernel(sparse_ids, varlen_ids, dense_vals, emb_sparse, emb_varlen, dense_weight):
    global _compiled, _last_res
    from concourse import bass_utils

    in_maps, extra = prepare_in_maps(sparse_ids, varlen_ids, dense_vals,
                                     emb_sparse, emb_varlen, dense_weight)
    if 1 not in _compiled:
        _compiled[1] = _build_nc()
    res = bass_utils.run_bass_kernel_spmd(_compiled[1], in_maps,
                                          core_ids=list(range(8)))
    _last_res = res
    return assemble_output(res.results, extra)


# revision 22
# speedup vs baseline: 1.0856x; 1.0856x over previous
"""DLRM-style embedding lookup kernel for 8 TRN2 NeuronCores.

Strategy: 4 table-shards x 2 row-shards. Core c = (varlen table k=c//2,
row half h=c%2) performs the 50-id varlen lookup+pool for its table over
half the batch on device; the dim-1 sparse-table lookups, the 13-dim
dense matvec, and ~0.02% placement-overflow fixups are exact host-side
terms added at assembly (the SWDGE dma_gather path is not usable under
this runtime, so those 16 lookups/row stay on host as in the original).

Device pipeline per core: bf16-pair varlen tables (map0 ‖ map1
encodings, 8192 u32/partition in SBUF). The host places each id into a
parity-free bin grid (8 groups x 7 slots, 2-choice + cuckoo eviction;
group = top-3 or low-3 id bits selects the map). Per 1024-row block: one
k-major ap_gather (7168 idxs — the gpsimd gather is the critical path at
~27ns/idx, so fewer fetched u32/row than the older 16-bin parity grid is
a direct win), PE matmul expands host-sent [1, c, c^2] code rows into
(code - partition)^2 per bf16 HALF in PSUM (codes are per-half: the
half not holding the slot-id carries the pad code), DVE builds the
(z == 0) keep-mask and multiplies it in place, and accumulating
ones-matmuls pool both halves of each slot into PSUM (region-major so
each PSUM bank has one open accumulation group at a time).
"""
import sys
sys.path.insert(0, "/opt/trn_rl_repo")

import numpy as np
import ml_dtypes
from contextlib import ExitStack

B = 65536
N_SPARSE = 16
N_VARLEN = 4
MAXLEN = 50
N_DENSE = 13
VOCAB = 1_000_000

RC = B // 2            # rows per core (2 row-shards)
NG = 8                 # partition groups == bins per row
KPB = 7                # u32 slots per (row, group) bin
RB = 1024              # rows per device block
NB = RC // RB          # 32 blocks
BLK = KPB * RB         # gather idxs per group per block = 7168
STREAM = KPB * RC      # 229376 per group
NE_TBL = 8192          # u32 per partition (map0 + map1)
PADC = 16              # pad code (never matches partition 0..15)

_compiled = {}
_last_res = None


# ---------------------------------------------------------------- device ---
def _build_nc(repeat=1):
    """repeat > 1 re-runs the whole per-block pipeline (used only by the
    timing harness to measure per-execution time as a slope in reps,
    cancelling the large axon dispatch overhead)."""
    import concourse.bass as bass
    import concourse.tile as tile
    from concourse import bacc, mybir

    nc = bacc.Bacc("TRN2", target_bir_lowering=False, debug=False)
    dt = mybir.dt
    tbl = nc.dram_tensor("tbl", [128, NE_TBL], dt.uint32, kind="ExternalInput").ap()
    didx = nc.dram_tensor("didx", [128, STREAM // 16], dt.int16, kind="ExternalInput").ap()
    cmat = nc.dram_tensor("cmat", [17, 2 * STREAM], dt.bfloat16, kind="ExternalInput").ap()
    w17 = nc.dram_tensor("w17", [17, 128], dt.bfloat16, kind="ExternalInput").ap()
    outv = nc.dram_tensor("outv", [NB, RB], dt.float32, kind="ExternalOutput").ap()

    CH = 512                        # bf16 halves per z / copy_pred chunk
    NCH = 2 * BLK // CH             # 28 chunks per block
    with tile.TileContext(nc) as tc:
        with ExitStack() as ctx:
            pconst = ctx.enter_context(tc.tile_pool(name="const", bufs=1))
            pidx = ctx.enter_context(tc.tile_pool(name="idx", bufs=3))
            pcm = ctx.enter_context(tc.tile_pool(name="cm", bufs=2))
            pgat = ctx.enter_context(tc.tile_pool(name="gat", bufs=3))
            pzbf = ctx.enter_context(tc.tile_pool(name="zbf", bufs=3))
            pstage = ctx.enter_context(tc.tile_pool(name="stage", bufs=2))
            pz = ctx.enter_context(tc.tile_pool(name="pz", bufs=3, space="PSUM"))
            po = ctx.enter_context(tc.tile_pool(name="po", bufs=2, space="PSUM"))

            t_tbl = pconst.tile([128, NE_TBL], dt.uint32)
            nc.sync.dma_start(t_tbl[:], tbl[:])
            t_w17 = pconst.tile([17, 128], dt.bfloat16)
            nc.sync.dma_start(t_w17[:], w17[:])
            t_ones = pconst.tile([128, 1], dt.bfloat16)
            nc.vector.memset(t_ones[:], 1.0)
            t_zero = pconst.tile([128, CH], dt.bfloat16)
            nc.vector.memset(t_zero[:], 0)

            def input_dma(b):
                t_di = pidx.tile([128, BLK // 16], dt.int16, tag="di")
                nc.sync.dma_start(t_di[:], didx[:, b * (BLK // 16):(b + 1) * (BLK // 16)])
                t_cm = pcm.tile([17, 2 * BLK], dt.bfloat16, tag="cm")
                nc.sync.dma_start(t_cm[:], cmat[:, b * 2 * BLK:(b + 1) * 2 * BLK])
                return t_di, t_cm

            for b in [b for _ in range(repeat) for b in range(NB)]:
                t_di, t_cm = input_dma(b)
                t_gat = pgat.tile([128, BLK, 1], dt.uint32, tag="g")
                nc.gpsimd.ap_gather(t_gat[:], t_tbl[:].unsqueeze(-1), t_di[:],
                                    channels=128, num_elems=NE_TBL, d=1, num_idxs=BLK)

                gat_bf = t_gat[:].squeeze(-1).bitcast(dt.bfloat16)  # [128, 2*BLK]
                # half-stream position f = s*2048 + rl*2 + h; chunk c of 512
                # covers (s = c//4, rl in [256*(c%4), 256*(c%4)+256), h in 0..1)
                view = gat_bf.rearrange("p (c x two) -> p c two x", c=NCH, two=2)

                t_po = po.tile([1, RB], dt.float32, tag="po")

                def emit_z(c):
                    t_pz = pz.tile([128, CH], dt.float32, tag="pz")
                    nc.tensor.matmul(t_pz[:], t_w17[:], t_cm[:, c * CH:(c + 1) * CH],
                                     start=True, stop=True)
                    # m = (z == 0) as bf16 1.0/0.0 — z is an exact integer
                    t_zb = pzbf.tile([128, CH], dt.bfloat16, tag="zb")
                    nc.vector.tensor_scalar(t_zb[:], t_pz[:], 0.0, None,
                                            mybir.AluOpType.is_equal)
                    return t_zb

                # process chunks region-major (all 7 slots of one po
                # 256-col region before the next) so each PSUM bank has at
                # most one open accumulation group at a time
                order = [s * 4 + rlb for rlb in range(4) for s in range(KPB)]
                zbuf = {c: emit_z(c) for c in order[:2]}
                for i, c in enumerate(order):
                    if i + 2 < NCH:
                        zbuf[order[i + 2]] = emit_z(order[i + 2])
                    t_zb = zbuf.pop(c)
                    nc.vector.tensor_tensor(
                        out=gat_bf[:, c * CH:(c + 1) * CH],
                        in0=gat_bf[:, c * CH:(c + 1) * CH],
                        in1=t_zb[:], op=mybir.AluOpType.mult)
                    s, rlb = c // 4, c % 4
                    first = (s == 0)
                    last = (s == KPB - 1)
                    for h in range(2):
                        nc.tensor.matmul(t_po[:, rlb * 256:(rlb + 1) * 256],
                                         t_ones[:], view[:, c, h, :],
                                         start=(first and h == 0),
                                         stop=(last and h == 1))
                t_st = pstage.tile([1, RB], dt.float32, tag="vst")
                nc.scalar.copy(t_st[:], t_po[:])
                nc.sync.dma_start(outv[b:b + 1, :], t_st[:])
    nc.compile()
    return nc


# ------------------------------------------------------------------ host ---
def _pack_u32(bf):  # bf16 [..., 2n] -> u32 [..., n]
    u = np.ascontiguousarray(bf).view(np.uint16).astype(np.uint32)
    return (u[..., 0::2] | (u[..., 1::2] << 16)).astype(np.uint32)


def _table_image(emb_varlen_k):
    """[128, 8192] u32: map0 ‖ map1 pair-packed bf16."""
    bf = ml_dtypes.bfloat16
    v = np.concatenate([emb_varlen_k, np.zeros(2**20 - VOCAB, emb_varlen_k.dtype)])
    v = v.astype(bf)
    m0 = v.reshape(8, 16, 8192).reshape(128, 8192)
    m1 = v.reshape(8192, 16, 8).transpose(2, 1, 0).reshape(128, 8192)
    return _pack_u32(np.concatenate([m0, m1], axis=1))


def _enc(ids, use_map1):
    """per-id (epair, code q, live half) for the chosen map. ids >= 0.
    map0: entry = (id>>13 partition, pair (id>>1)&4095, half id&1)
    map1: entry = (partition (id&7)*16 + (id>>3)&15, pair id>>8, half (id>>7)&1)
    """
    e0 = ((ids >> 1) & 4095).astype(np.int16)
    c0 = ((ids >> 13) & 15).astype(np.int16)
    h0 = (ids & 1).astype(np.int16)
    e1 = (4096 + (ids >> 8)).astype(np.int16)
    c1 = ((ids >> 3) & 15).astype(np.int16)
    h1 = ((ids >> 7) & 1).astype(np.int16)
    return (np.where(use_map1, e1, e0), np.where(use_map1, c1, c0),
            np.where(use_map1, h1, h0))


def _place_core(vl):
    """vl [R, 50] varlen ids. Parity-free 2-choice + cuckoo into a
    [R, 8 bins, 7 slots] grid. Bin g==(id>>17)&7 -> map0 placement;
    bin g==id&7 -> map1 (if both, map0). Returns grid_id [R,8,7] int64
    (-1 empty), grid_m1 [R,8,7] bool, overflow (rows, ids)."""
    R = vl.shape[0]
    rows = np.arange(R)
    cnt = np.zeros((R, NG), np.int32)
    grid_id = np.full((R, NG, KPB), -1, np.int64)
    placed = np.zeros((R, 50), bool)
    valid_all = np.zeros((R, 50), bool)
    ids_all = vl.astype(np.int64)
    b0s = ((ids_all >> 17) & 7).astype(np.int32)
    b1s = (ids_all & 7).astype(np.int32)

    for j in range(50):
        ids = ids_all[:, j]
        valid = ids > 0
        b0 = b0s[:, j]; b1 = b1s[:, j]
        valid_all[:, j] = valid
        c0 = cnt[rows, b0]; c1 = cnt[rows, b1]
        use0 = c0 <= c1
        ba = np.where(use0, b0, b1); bb = np.where(use0, b1, b0)
        sa = cnt[rows, ba]
        ok_a = valid & (sa < KPB)
        ra = rows[ok_a]
        grid_id[ra, ba[ok_a], sa[ok_a]] = ids[ok_a]
        cnt[ra, ba[ok_a]] += 1
        placed[ok_a, j] = True
        rem = valid & ~ok_a
        sb = cnt[rows, bb]
        ok_b = rem & (sb < KPB)
        rb = rows[ok_b]
        grid_id[rb, bb[ok_b], sb[ok_b]] = ids[ok_b]
        cnt[rb, bb[ok_b]] += 1
        placed[ok_b, j] = True

    # vectorized cuckoo eviction: rows independent; resolve one overflow
    # item per row per round.
    ovf = valid_all & ~placed
    for _ in range(8):
        has = ovf.any(axis=1)
        if not has.any():
            break
        r_sel = np.nonzero(has)[0]
        j_sel = ovf[r_sel].argmax(axis=1)
        ids = ids_all[r_sel, j_sel]
        bopt = np.stack([b0s[r_sel, j_sel], b1s[r_sel, j_sel]], 1)
        done = np.zeros(len(r_sel), bool)
        for bi in range(2):
            b = bopt[:, bi]
            for s in range(KPB):
                occ = grid_id[r_sel, b, s]
                o_b0 = ((occ >> 17) & 7).astype(np.int32)
                o_b1 = (occ & 7).astype(np.int32)
                alt = np.where(o_b0 == b, o_b1, o_b0)
                can = (~done) & (occ >= 0) & (cnt[r_sel, alt] < KPB)
                rr = r_sel[can]
                t = cnt[rr, alt[can]]
                grid_id[rr, alt[can], t] = occ[can]
                cnt[rr, alt[can]] += 1
                grid_id[rr, b[can], s] = ids[can]
                ovf[rr, j_sel[can]] = False
                done |= can
            if done.all():
                break

    orows, ojs = np.nonzero(ovf)
    g = np.arange(NG)[None, :, None]
    gid = grid_id
    is_b0 = ((gid >> 17) & 7) == g
    grid_m1 = (gid >= 0) & ~is_b0
    return grid_id, grid_m1, (orows, ids_all[orows, ojs])


def prepare_in_maps(sparse_ids, varlen_ids, dense_vals, emb_sparse, emb_varlen,
                    dense_weight):
    sparse_ids = np.asarray(sparse_ids)
    varlen_ids = np.asarray(varlen_ids)
    dense_vals = np.asarray(dense_vals, np.float32)
    emb_sparse = np.asarray(emb_sparse, np.float32)
    emb_varlen = np.asarray(emb_varlen, np.float32)
    dense_weight = np.asarray(dense_weight, np.float32)
    bf = ml_dtypes.bfloat16

    w17 = np.zeros((17, 128), np.float32)
    p16 = np.arange(128) % 16
    pg = np.arange(128) // 16
    w17[0] = (p16 ** 2)
    for g in range(NG):
        w17[1 + 2 * g] = np.where(pg == g, -2.0 * p16, 0.0)
        w17[2 + 2 * g] = np.where(pg == g, 1.0, 0.0)

    in_maps = []
    fixups = np.zeros(B, np.float64)
    for c in range(8):
        k, h = c // 2, c % 2
        r0 = h * RC
        vl = varlen_ids[r0:r0 + RC, k, :]
        gid, gm1, (fr, fi) = _place_core(vl)
        if len(fr):
            np.add.at(fixups, r0 + fr,
                      emb_varlen[k][fi].astype(bf).astype(np.float64))

        ge, gq, gh = _enc(np.maximum(gid, 0), gm1)
        empty = gid < 0
        ge = np.where(empty, 0, ge).astype(np.int16)
        # sort each bin's slots by table offset: slot s then holds the s-th
        # order statistic, so consecutive rows' same-slot offsets are
        # quantile-aligned — the gather ucode pairs reads two idxs apart in
        # the row-minor stream, and nearby addresses measure ~20% faster
        # than random on HW (any slot permutation is valid)
        srt = np.argsort(ge, axis=2, kind="stable")
        ge = np.take_along_axis(ge, srt, axis=2)
        gq = np.take_along_axis(gq, srt, axis=2)
        gh = np.take_along_axis(gh, srt, axis=2)
        empty = np.take_along_axis(empty, srt, axis=2)
        # per-half codes [R, 8, 7, 2]: live half gets q, other half PADC
        codes = np.full(gid.shape + (2,), PADC, np.int16)
        live = np.where(empty, PADC, gq).astype(np.int16)
        hsel = np.clip(gh, 0, 1).astype(np.int64)
        ridx, gidx, sidx = np.indices(gid.shape)
        codes[ridx, gidx, sidx, hsel] = live
        codes[empty] = PADC

        # [R,8,7] -> stream [g, pos]: pos = blk*7168 + s*1024 + rl
        def to_stream(a):
            x = a.reshape(NB, RB, NG, KPB)          # blk, rl, g, s
            x = x.transpose(2, 0, 3, 1)             # g, blk, s, rl
            return np.ascontiguousarray(x).reshape(NG, STREAM)

        estream = to_stream(ge)

        # codes [R,8,7,2] -> [g, blk, s, rl, h] -> [g, 2*STREAM]
        xc = codes.reshape(NB, RB, NG, KPB, 2).transpose(2, 0, 3, 1, 4)
        cstream = np.ascontiguousarray(xc).reshape(NG, 2 * STREAM).astype(np.float32)

        didx = np.empty((128, STREAM // 16), np.int16)
        for g in range(NG):
            t = estream[g].reshape(NB, BLK // 16, 16).transpose(0, 2, 1)
            didx[16 * g:16 * g + 16] = t.transpose(1, 0, 2).reshape(16, STREAM // 16)
        cmat = np.empty((17, 2 * STREAM), np.float32)
        cmat[0] = 1.0
        for g in range(NG):
            cmat[1 + 2 * g] = cstream[g]
            cmat[2 + 2 * g] = cstream[g] ** 2

        in_maps.append(dict(
            tbl=_table_image(emb_varlen[k]),
            didx=didx,
            cmat=cmat.astype(bf),
            w17=w17.astype(bf)))

    # host-side terms: sparse lookups + dense matvec (exact f64)
    tbl_s = np.arange(N_SPARSE)[None, :]
    sparse = emb_sparse.astype(np.float64)[tbl_s, sparse_ids].sum(axis=1)
    dense = (dense_vals.astype(np.float64) @ dense_weight.astype(np.float64)).reshape(B)
    return in_maps, fixups + sparse + dense


def assemble_output(results, extra):
    out = np.zeros(B, np.float64)
    for c in range(8):
        k, h = c // 2, c % 2
        r0 = h * RC
        out[r0:r0 + RC] += results[c]["outv"].reshape(RC).astype(np.float64)
    out += extra
    return out.astype(np.float32).reshape(B, 1)


def kernel(sparse_ids, varlen_ids, dense_vals, emb_sparse, emb_varlen, dense_weight):
    global _compiled, _last_res
    from concourse import bass_utils

    in_maps, extra = prepare_in_maps(sparse_ids, varlen_ids, dense_vals,
                                     emb_sparse, emb_varlen, dense_weight)
    if 1 not in _compiled:
        _compiled[1] = _build_nc()
    res = bass_utils.run_bass_kernel_spmd(_compiled[1], in_maps,
                                          core_ids=list(range(8)))
    _last_res = res
    return assemble_output(res.results, extra)
